# revision 35
# baseline (speedup 1.0000x reference)
"""LlamaPEER MoE-routing kernel for 8 NeuronCores (TRN2, Bass/Tile).

Data-parallel over B*T (2048 tokens -> 256/core); expert tables replicated
as one host-packed fp32 [E, 2D] table (down|up rows concatenated), so each
(token, slot) needs ONE 16KB-descriptor indirect gather. Per core:
  1. qT = Wq^T @ x^T on PE (fp32, routing bit-identical to reference).
  2. Per (half, head) chain: sims on PE, top-8 via DVE max/max_index,
     K x K cross combine + second top-8 + iota-mask index extraction.
  3. Per slot: gather cat row; down-dot via DVE mult + Scalar accumulate;
     silu (Scalar) * relu(score); diag build (DVE); up-proj via PE diag
     matmuls accumulated in PSUM; one copy-out + store per 128-token half.
Emission interleaves routing/gather/consume chains so the gather stream
starts ~40us in and the 16 DMA queues stay saturated.
"""

import numpy as np

import concourse.bass as bass
import concourse.tile as tile
from concourse import mybir
from concourse.bass_utils import run_bass_kernel_spmd
from concourse.vector_clock import ScopedClock

N_CORES = 8
B, T, D = 2, 1024, 2048
H, K, DK = 4, 8, 64
E = 16384
NK = 128
TOK = (B * T) // N_CORES  # 256 tokens per core
NSLOT = H * K  # 32 slots per 128-token half
CAT = 2 * D  # concatenated row length
FP = mybir.dt.float32
I32 = mybir.dt.int32
U32 = mybir.dt.uint32

# --- workaround: this walrus build allows only 1 sync-wait command on the
# final SP drain; split the tile-context drain into 1-wait drains.
_MAX_DRAIN_WAITS = 1


def _patched_drain_and_barrier(self, tick_clock, wait_clock):
    nc = self.nc
    drain_inst = nc.sync.drain()
    wait_clock.add_sem_waits(
        drain_inst.ins, ScopedClock({None: tick_clock.global_clock})
    )
    si = drain_inst.ins.sync_info
    if si is not None and len(si.on_wait) > _MAX_DRAIN_WAITS:
        waits = list(si.on_wait)
        upds = list(si.on_update)
        drain_inst.ins.sync_info = mybir.SyncInfo(
            on_wait=waits[:_MAX_DRAIN_WAITS], on_update=[]
        )
        rest = waits[_MAX_DRAIN_WAITS:]
        while rest:
            extra = nc.sync.drain()
            extra.ins.sync_info = mybir.SyncInfo(
                on_wait=rest[:_MAX_DRAIN_WAITS],
                on_update=upds if len(rest) <= _MAX_DRAIN_WAITS else [],
            )
            rest = rest[_MAX_DRAIN_WAITS:]
    nc.all_engine_barrier()
    popped = nc._tile_sem_poison_stack.pop()
    assert popped is self._sem_poison
    all_sems = list(self.sems.allocated().values())
    for i in range(0, len(all_sems), 8):
        nc.clear_and_free_semaphores(all_sems[i : i + 8])
    nc.all_engine_barrier()


tile.TileContext._drain_and_barrier = _patched_drain_and_barrier

_orig_lower_ordered = tile.TileContext._lower_ordered_insts


def _patched_lower_ordered(self, postordered_blocks):
    # this walrus build supports only one sync-wait command per instruction:
    # hoist extra waits onto same-engine NoOps placed just before.
    for bb_name, insts in postordered_blocks.items():
        new = []
        for inst in insts:
            si = getattr(inst, "sync_info", None)
            eng = getattr(inst, "engine", None)
            if si is not None and eng is not None and len(si.on_wait) > 1:
                waits = list(si.on_wait)
                for w in waits[:-1]:
                    nop = mybir.InstNoOp(
                        name=self.nc.get_next_instruction_name(),
                        sync_info=mybir.SyncInfo(on_wait=[w], on_update=[]),
                        bass_nofuse=True,
                        engine=eng,
                    )
                    new.append(nop)
                inst.sync_info = mybir.SyncInfo(
                    on_wait=[waits[-1]], on_update=list(si.on_update)
                )
            new.append(inst)
        insts[:] = new
    return _orig_lower_ordered(self, postordered_blocks)


tile.TileContext._lower_ordered_insts = _patched_lower_ordered


def _re(ap, dims):
    """Return ap with its free-axis access pattern replaced by `dims`
    (list of [step, count]); keeps the partition dim."""
    return ap.__replace__(ap=[list(ap.ap)[0]] + [list(d) for d in dims])


def build_program():
    nc = bass.Bass("TRN2", target_bir_lowering=False, debug=False)

    NDCH = D // 128  # 16 d-chunks

    # xts/wqs are host-prelayouted to the exact SBUF image (contiguous
    # 16KB-per-partition loads): xts[p, c*TOK+t] = x[t, c*128+p];
    # wqs[p, m*16*128 + c*128 + o] = Wq[c*128+p, m*128+o] (m-major, so the
    # 1MB slice feeding qT chunk m=0 loads first and routing starts early).
    xts_d = nc.dram_tensor("xts", [128, NDCH * TOK], FP, kind="ExternalInput")
    x_d = nc.dram_tensor("x", [TOK, D], FP, kind="ExternalInput")
    wqs_d = nc.dram_tensor("wqs", [128, NDCH * 512], FP, kind="ExternalInput")
    kt_d = nc.dram_tensor("keyst", [2 * DK, H * NK], FP, kind="ExternalInput")
    ed_d = nc.dram_tensor("e_down", [E, D], FP, kind="ExternalInput")
    eu_d = nc.dram_tensor("e_up", [E, D], FP, kind="ExternalInput")
    id_d = nc.dram_tensor("ident", [128, 128], FP, kind="ExternalInput")
    io_d = nc.dram_tensor("iota64", [128, 64], FP, kind="ExternalInput")
    out_d = nc.dram_tensor("out", [TOK, D], FP, kind="ExternalOutput")

    with tile.TileContext(nc) as tc:
        with (
            tc.tile_pool(name="const", bufs=1) as cpool,
            tc.tile_pool(name="mats", bufs=1) as mpool,
            tc.tile_pool(name="route", bufs=3) as rpool,
            tc.tile_pool(name="persist", bufs=1) as ppool,
            tc.tile_pool(name="gd", bufs=3) as gdpool,
            tc.tile_pool(name="gu", bufs=8) as gupool,
            tc.tile_pool(name="scr", bufs=2) as spool,
            tc.tile_pool(name="dg", bufs=4) as dgpool,
            tc.tile_pool(name="ob", bufs=2) as opool,
            tc.tile_pool(name="psqt", bufs=2, space="PSUM") as psqt,
            tc.tile_pool(name="pssim", bufs=2, space="PSUM") as pssim,
            tc.tile_pool(name="psacc", bufs=1, space="PSUM") as psacc,
        ):
            wq_sb = mpool.tile([128, NDCH * 512], FP)
            MW = NDCH * 128  # 2048 columns per m-chunk of wq (m-major)
            # consts + the m=0 wq slice + xt load first; x and the remaining
            # wq slices trail BEHIND the first gathers so the DMA queues
            # never idle between the load phase and the gather stream.
            ident = cpool.tile([128, 128], FP)
            nc.sync.dma_start(ident[:], id_d.ap())
            iota = cpool.tile([128, 64], FP)
            nc.sync.dma_start(iota[:], io_d.ap())
            kt_sb = cpool.tile([2 * DK, H * NK], FP)
            nc.sync.dma_start(kt_sb[:], kt_d.ap())
            nc.sync.dma_start(wq_sb[:, 0:MW], wqs_d.ap()[:, 0:MW])
            xt_sb = mpool.tile([128, NDCH * TOK], FP)
            # quartered so qT chunk matmuls overlap the load stream
            for q4 in range(4):
                nc.sync.dma_start(
                    xt_sb[:, q4 * 4 * TOK : (q4 + 1) * 4 * TOK],
                    xts_d.ap()[:, q4 * 4 * TOK : (q4 + 1) * 4 * TOK],
                )

            # PE warmup: back-to-back matmuls on the first wq slice release
            # the HAM clock throttle while the big loads stream in, so the
            # latency-critical qT chain runs at full clock.
            wps = pssim.tile([128, 128], FP, tag="ps", name="warm")
            for w in range(12):
                nc.tensor.matmul(
                    wps[:, 0:128], lhsT=wq_sb[:, 0:128], rhs=wq_sb[:, 128:256],
                    start=(w == 0), stop=(w == 11),
                )
            wsb = rpool.tile([128, 128], FP, tag="sim", name="warmout")
            nc.scalar.activation(wsb[:], wps[:, 0:128], mybir.ActivationFunctionType.Copy)

            # trailing loads (consumed from the first consume chain on)
            x_sb = []
            for hf in range(2):
                xh = ppool.tile([128, D], FP, tag=f"x{hf}", name=f"x{hf}")
                nc.sync.dma_start(xh[:], x_d.ap()[hf * 128 : hf * 128 + 128, :])
                x_sb.append(xh)
            for m in range(1, 4):
                nc.sync.dma_start(
                    wq_sb[:, m * MW : (m + 1) * MW], wqs_d.ap()[:, m * MW : (m + 1) * MW]
                )

            qt_sb = ppool.tile([128, 4 * TOK], FP)
            fi_all = [
                ppool.tile([128, NSLOT], I32, tag=f"fi{hf}", name=f"fi{hf}")
                for hf in range(2)
            ]
            fsr_all = [
                ppool.tile([128, NSLOT], FP, tag=f"fsr{hf}", name=f"fsr{hf}")
                for hf in range(2)
            ]
            hid_all = [
                ppool.tile([128, NSLOT], FP, tag=f"hid{hf}", name=f"hid{hf}")
                for hf in range(2)
            ]
            hs2_all = [
                ppool.tile([128, NSLOT], FP, tag=f"hs2{hf}", name=f"hs2{hf}")
                for hf in range(2)
            ]
            acc = {}
            gts = {}

            def emit_qt(m):
                # qT chunk m: psum_q[p, t] = q[t, m*128+p] for all 256 tokens
                pq = psqt.tile([128, TOK], FP, tag="pq", name=f"pq{m}")
                for c in range(NDCH):
                    nc.tensor.matmul(
                        pq[:],
                        lhsT=wq_sb[:, m * MW + c * 128 : m * MW + (c + 1) * 128],
                        rhs=xt_sb[:, c * TOK : (c + 1) * TOK],
                        start=(c == 0),
                        stop=(c == NDCH - 1),
                    )
                nc.scalar.activation(
                    qt_sb[:, m * TOK : (m + 1) * TOK],
                    pq[:],
                    mybir.ActivationFunctionType.Copy,
                )

            def emit_route(hf, h):
                t0 = hf * 128
                ss = []
                ii = []
                for sub in range(2):
                    ps = pssim.tile([128, NK], FP, tag="ps")
                    nc.tensor.matmul(
                        ps[:],
                        lhsT=qt_sb[
                            sub * 64 : (sub + 1) * 64,
                            h * TOK + t0 : h * TOK + t0 + 128,
                        ],
                        rhs=kt_sb[sub * 64 : (sub + 1) * 64, h * NK : (h + 1) * NK],
                        start=True,
                        stop=True,
                    )
                    sim = rpool.tile([128, NK], FP, tag="sim")
                    nc.scalar.activation(
                        sim[:], ps[:], mybir.ActivationFunctionType.Copy
                    )
                    s = rpool.tile([128, 8], FP, tag="s")
                    nc.vector.max(s[:], sim[:])
                    idx = rpool.tile([128, 8], U32, tag="idx")
                    nc.vector.max_index(idx[:], s[:], sim[:])
                    idf = rpool.tile([128, 8], FP, tag="idf")
                    nc.vector.tensor_copy(idf[:], idx[:])
                    ss.append(s)
                    ii.append(idf)
                # cross combine: [128, 8(k1), 8(k2)]
                alls = rpool.tile([128, 64], FP, tag="alls")
                a3 = _re(alls[:], [[8, 8], [1, 8]])
                nc.vector.tensor_tensor(
                    out=a3,
                    in0=_re(ss[0][:], [[1, 8], [0, 8]]),
                    in1=_re(ss[1][:], [[0, 8], [1, 8]]),
                    op=mybir.AluOpType.add,
                )
                alli = rpool.tile([128, 64], FP, tag="alli")
                ai3 = _re(alli[:], [[8, 8], [1, 8]])
                nc.vector.tensor_scalar(
                    out=ai3,
                    in0=_re(ii[0][:], [[1, 8], [0, 8]]),
                    scalar1=float(NK),
                    scalar2=None,
                    op0=mybir.AluOpType.mult,
                )
                nc.vector.tensor_tensor(
                    out=ai3,
                    in0=ai3,
                    in1=_re(ii[1][:], [[0, 8], [1, 8]]),
                    op=mybir.AluOpType.add,
                )
                fs = rpool.tile([128, 8], FP, tag="fs")
                nc.vector.max(fs[:], alls[:])
                pk = rpool.tile([128, 8], U32, tag="pk")
                nc.vector.max_index(pk[:], fs[:], alls[:])
                pkf = rpool.tile([128, 8], FP, tag="pkf")
                nc.vector.tensor_copy(pkf[:], pk[:])
                # scores: relu on scalar engine
                nc.scalar.activation(
                    fsr_all[hf][:, h * 8 : (h + 1) * 8],
                    fs[:],
                    mybir.ActivationFunctionType.Relu,
                )
                # mask[p, j, n] = (pk[p,j] == iota[p,n]) * alli[p,n]; reduce
                mask = rpool.tile([128, 512], FP, tag="mask", bufs=2)
                m3 = _re(mask[:], [[64, 8], [1, 64]])
                nc.vector.tensor_tensor(
                    out=m3,
                    in0=_re(pkf[:], [[1, 8], [0, 64]]),
                    in1=_re(iota[:], [[0, 8], [1, 64]]),
                    op=mybir.AluOpType.is_equal,
                )
                nc.vector.tensor_tensor(
                    out=m3,
                    in0=m3,
                    in1=_re(alli[:], [[0, 8], [1, 64]]),
                    op=mybir.AluOpType.mult,
                )
                fif = rpool.tile([128, 8], FP, tag="fif")
                nc.vector.tensor_reduce(
                    fif[:],
                    m3,
                    axis=mybir.AxisListType.X,
                    op=mybir.AluOpType.add,
                )
                nc.vector.tensor_copy(fi_all[hf][:, h * 8 : (h + 1) * 8], fif[:])

            def emit_gather(hf, h):
                # For the very last chain, issue all down-gathers first: the
                # dot/silu chain then finishes while the up rows stream in,
                # so the exposed tail is just the final matmul + store.
                down_first = hf == 1 and h == H - 1
                gds = []
                for j in range(K):
                    k = h * 8 + j
                    gd = gdpool.tile([128, D], FP, tag="gd")
                    nc.gpsimd.indirect_dma_start(
                        out=gd[:],
                        out_offset=None,
                        in_=ed_d.ap(),
                        in_offset=bass.IndirectOffsetOnAxis(
                            ap=fi_all[hf][:, k : k + 1], axis=0
                        ),
                    )
                    gds.append(gd)
                    if not down_first:
                        gu = gupool.tile([128, D], FP, tag="gu")
                        nc.gpsimd.indirect_dma_start(
                            out=gu[:],
                            out_offset=None,
                            in_=eu_d.ap(),
                            in_offset=bass.IndirectOffsetOnAxis(
                                ap=fi_all[hf][:, k : k + 1], axis=0
                            ),
                        )
                        gts.setdefault((hf, h), []).append((gd, gu))
                if down_first:
                    for j in range(K):
                        k = h * 8 + j
                        gu = gupool.tile([128, D], FP, tag="gu")
                        nc.gpsimd.indirect_dma_start(
                            out=gu[:],
                            out_offset=None,
                            in_=eu_d.ap(),
                            in_offset=bass.IndirectOffsetOnAxis(
                                ap=fi_all[hf][:, k : k + 1], axis=0
                            ),
                        )
                        gts.setdefault((hf, h), []).append((gds[j], gu))

            def emit_consume(hf, h):
                t0 = hf * 128
                if h == 0:
                    acc[hf] = psacc.tile([128, D], FP, tag="acc", name=f"acc{hf}")
                # sub-groups of 2 slots: dots stream on DVE (scalar accums
                # trail by one slot), then a small silu/hs2 batch and the
                # dg+matmuls, so gather buffers release continuously and the
                # final group drains fast at the end of the stream.  The very
                # last chain ends in two 1-slot groups to minimize the tail.
                last = hf == 1 and h == H - 1
                groups = [(0, 2), (2, 2), (4, 2), (6, 1), (7, 1)] if last else [
                    (0, 2), (2, 2), (4, 2), (6, 2)
                ]
                for j0, glen in groups:
                    for j in range(j0, j0 + glen):
                        k = h * 8 + j
                        gd, gu = gts[(hf, h)][j]
                        scr = spool.tile([128, D], FP, tag="scr", bufs=3)
                        nc.vector.tensor_tensor(
                            out=scr[:],
                            in0=gd[:],
                            in1=x_sb[hf][:],
                            op=mybir.AluOpType.mult,
                        )
                        scr2 = spool.tile([128, D], FP, tag="scr2", bufs=1)
                        nc.scalar.activation(
                            scr2[:],
                            scr[:],
                            mybir.ActivationFunctionType.Copy,
                            accum_out=hid_all[hf][:, k : k + 1],
                        )
                    k0 = h * 8 + j0
                    hsil4 = rpool.tile([128, 2], FP, tag="hsil", padded_shape=[128, 2])
                    nc.scalar.activation(
                        hsil4[:, 0:glen],
                        hid_all[hf][:, k0 : k0 + glen],
                        mybir.ActivationFunctionType.Silu,
                    )
                    nc.vector.tensor_tensor(
                        out=hs2_all[hf][:, k0 : k0 + glen],
                        in0=hsil4[:, 0:glen],
                        in1=fsr_all[hf][:, k0 : k0 + glen],
                        op=mybir.AluOpType.mult,
                    )
                    for j in range(j0, j0 + glen):
                        k = h * 8 + j
                        gd, gu = gts[(hf, h)][j]
                        dg = dgpool.tile([128, 128], FP, tag="dg")
                        nc.vector.tensor_scalar_mul(
                            dg[:], ident[:], hs2_all[hf][:, k : k + 1]
                        )
                        for c4 in range(4):
                            nc.tensor.matmul(
                                acc[hf][:, c4 * 512 : (c4 + 1) * 512],
                                lhsT=dg[:],
                                rhs=gu[:, c4 * 512 : (c4 + 1) * 512],
                                start=(k == 0),
                                stop=(k == NSLOT - 1),
                            )
                if h == H - 1:
                    # chunked copy-out overlaps the tail matmuls and halves
                    # the exposed drain at the end of each half.
                    for c4 in range(4):
                        obc = opool.tile([128, 512], FP, tag="obc")
                        nc.scalar.activation(
                            obc[:],
                            acc[hf][:, c4 * 512 : (c4 + 1) * 512],
                            mybir.ActivationFunctionType.Copy,
                        )
                        nc.sync.dma_start(
                            out_d.ap()[t0 : t0 + 128, c4 * 512 : (c4 + 1) * 512],
                            obc[:],
                        )

            # Chains c=0..7 -> (hf, h) = (c // 4, c % 4).  Routing stays one
            # chain ahead of consumption; gathers are enqueued early and
            # self-pace against gather-buffer releases (nothing else runs on
            # gpsimd, so SWDGE stalls are harmless).  qt chunks sit in PE
            # idle gaps one full window before the routing that needs them.
            CH = [(0, 0), (0, 1), (0, 2), (0, 3), (1, 0), (1, 1), (1, 2), (1, 3)]
            emit_qt(0)
            emit_route(*CH[0])
            emit_gather(*CH[0])
            emit_qt(1)
            emit_route(*CH[1])
            emit_gather(*CH[1])
            emit_qt(2)
            emit_route(*CH[2])
            emit_gather(*CH[2])
            emit_consume(*CH[0])
            emit_qt(3)
            emit_route(*CH[3])
            emit_gather(*CH[3])
            emit_consume(*CH[1])
            emit_route(*CH[4])
            emit_gather(*CH[4])
            emit_consume(*CH[2])
            emit_route(*CH[5])
            emit_gather(*CH[5])
            emit_consume(*CH[3])
            emit_route(*CH[6])
            emit_gather(*CH[6])
            emit_consume(*CH[4])
            emit_route(*CH[7])
            emit_gather(*CH[7])
            emit_consume(*CH[5])
            emit_consume(*CH[6])
            emit_consume(*CH[7])

    return nc


_CACHED = {}


def kernel(x, Wq, keys, e_down, e_up):
    x = np.asarray(x, dtype=np.float32)
    Wq = np.asarray(Wq, dtype=np.float32)
    keys = np.asarray(keys, dtype=np.float32)
    e_down = np.asarray(e_down, dtype=np.float32)
    e_up = np.asarray(e_up, dtype=np.float32)

    if "nc" not in _CACHED:
        _CACHED["nc"] = build_program()
    nc = _CACHED["nc"]

    xf = x.reshape(B * T, D)
    keyst = np.ascontiguousarray(keys.transpose(2, 3, 0, 1)).reshape(2 * DK, H * NK)
    # keyst[sub*64+dk, h*NK + nk] = keys[h, nk, sub, dk]
    ident = np.eye(128, dtype=np.float32)
    iota64 = np.tile(np.arange(64, dtype=np.float32), (128, 1))
    NDCH = D // 128
    # wqs[p, m*16*128 + c*128 + o] = Wq[c*128+p, m*128+o] (m-major)
    wqs = np.ascontiguousarray(
        Wq.reshape(NDCH, 128, 4, 128).transpose(1, 2, 0, 3).reshape(128, NDCH * 512)
    )

    in_maps = []
    for c in range(N_CORES):
        xs = np.ascontiguousarray(xf[c * TOK : (c + 1) * TOK])
        # xts[p, ch*TOK+t] = xs[t, ch*128+p]
        xts = np.ascontiguousarray(
            xs.reshape(TOK, NDCH, 128).transpose(2, 1, 0).reshape(128, NDCH * TOK)
        )
        in_maps.append(
            {
                "x": xs,
                "xts": xts,
                "wqs": wqs,
                "keyst": keyst,
                "e_down": e_down,
                "e_up": e_up,
                "ident": ident,
                "iota64": iota64,
            }
        )

    res = run_bass_kernel_spmd(nc, in_maps, core_ids=list(range(N_CORES)))
    _CACHED["res"] = res
    out = np.concatenate([res.results[c]["out"] for c in range(N_CORES)], axis=0)
    return out.reshape(B, T, D)


# revision 37
# speedup vs baseline: 1.0054x; 1.0054x over previous
"""LlamaPEER MoE-routing kernel for 8 NeuronCores (TRN2, Bass/Tile).

Data-parallel over B*T (2048 tokens -> 256/core); expert tables replicated
as one host-packed fp32 [E, 2D] table (down|up rows concatenated), so each
(token, slot) needs ONE 16KB-descriptor indirect gather. Per core:
  1. qT = Wq^T @ x^T on PE (fp32, routing bit-identical to reference).
  2. Per (half, head) chain: sims on PE, top-8 via DVE max/max_index,
     K x K cross combine + second top-8 + iota-mask index extraction.
  3. Per slot: gather cat row; down-dot via DVE mult + Scalar accumulate;
     silu (Scalar) * relu(score); diag build (DVE); up-proj via PE diag
     matmuls accumulated in PSUM; one copy-out + store per 128-token half.
Emission interleaves routing/gather/consume chains so the gather stream
starts ~40us in and the 16 DMA queues stay saturated.
"""

import numpy as np

import concourse.bass as bass
import concourse.tile as tile
from concourse import mybir
from concourse.bass_utils import run_bass_kernel_spmd
from concourse.vector_clock import ScopedClock

N_CORES = 8
B, T, D = 2, 1024, 2048
H, K, DK = 4, 8, 64
E = 16384
NK = 128
TOK = (B * T) // N_CORES  # 256 tokens per core
NSLOT = H * K  # 32 slots per 128-token half
CAT = 2 * D  # concatenated row length
FP = mybir.dt.float32
I32 = mybir.dt.int32
U32 = mybir.dt.uint32

# --- workaround: this walrus build allows only 1 sync-wait command on the
# final SP drain; split the tile-context drain into 1-wait drains.
_MAX_DRAIN_WAITS = 1


def _patched_drain_and_barrier(self, tick_clock, wait_clock):
    nc = self.nc
    drain_inst = nc.sync.drain()
    wait_clock.add_sem_waits(
        drain_inst.ins, ScopedClock({None: tick_clock.global_clock})
    )
    si = drain_inst.ins.sync_info
    if si is not None and len(si.on_wait) > _MAX_DRAIN_WAITS:
        waits = list(si.on_wait)
        upds = list(si.on_update)
        drain_inst.ins.sync_info = mybir.SyncInfo(
            on_wait=waits[:_MAX_DRAIN_WAITS], on_update=[]
        )
        rest = waits[_MAX_DRAIN_WAITS:]
        while rest:
            extra = nc.sync.drain()
            extra.ins.sync_info = mybir.SyncInfo(
                on_wait=rest[:_MAX_DRAIN_WAITS],
                on_update=upds if len(rest) <= _MAX_DRAIN_WAITS else [],
            )
            rest = rest[_MAX_DRAIN_WAITS:]
    nc.all_engine_barrier()
    popped = nc._tile_sem_poison_stack.pop()
    assert popped is self._sem_poison
    all_sems = list(self.sems.allocated().values())
    for i in range(0, len(all_sems), 8):
        nc.clear_and_free_semaphores(all_sems[i : i + 8])
    nc.all_engine_barrier()


tile.TileContext._drain_and_barrier = _patched_drain_and_barrier

_orig_lower_ordered = tile.TileContext._lower_ordered_insts


def _patched_lower_ordered(self, postordered_blocks):
    # this walrus build supports only one sync-wait command per instruction:
    # hoist extra waits onto same-engine NoOps placed just before.
    for bb_name, insts in postordered_blocks.items():
        new = []
        for inst in insts:
            si = getattr(inst, "sync_info", None)
            eng = getattr(inst, "engine", None)
            if si is not None and eng is not None and len(si.on_wait) > 1:
                waits = list(si.on_wait)
                for w in waits[:-1]:
                    nop = mybir.InstNoOp(
                        name=self.nc.get_next_instruction_name(),
                        sync_info=mybir.SyncInfo(on_wait=[w], on_update=[]),
                        bass_nofuse=True,
                        engine=eng,
                    )
                    new.append(nop)
                inst.sync_info = mybir.SyncInfo(
                    on_wait=[waits[-1]], on_update=list(si.on_update)
                )
            new.append(inst)
        insts[:] = new
    return _orig_lower_ordered(self, postordered_blocks)


tile.TileContext._lower_ordered_insts = _patched_lower_ordered


def _re(ap, dims):
    """Return ap with its free-axis access pattern replaced by `dims`
    (list of [step, count]); keeps the partition dim."""
    return ap.__replace__(ap=[list(ap.ap)[0]] + [list(d) for d in dims])


def build_program():
    nc = bass.Bass("TRN2", target_bir_lowering=False, debug=False)

    NDCH = D // 128  # 16 d-chunks

    # xts/wqs are host-prelayouted to the exact SBUF image (contiguous
    # 16KB-per-partition loads): xts[p, c*TOK+t] = x[t, c*128+p];
    # wqs[p, m*16*128 + c*128 + o] = Wq[c*128+p, m*128+o] (m-major, so the
    # 1MB slice feeding qT chunk m=0 loads first and routing starts early).
    xts_d = nc.dram_tensor("xts", [128, NDCH * TOK], FP, kind="ExternalInput")
    x_d = nc.dram_tensor("x", [TOK, D], FP, kind="ExternalInput")
    wqs_d = nc.dram_tensor("wqs", [128, NDCH * 512], FP, kind="ExternalInput")
    kt_d = nc.dram_tensor("keyst", [2 * DK, H * NK], FP, kind="ExternalInput")
    ed_d = nc.dram_tensor("e_down", [E, D], FP, kind="ExternalInput")
    eu_d = nc.dram_tensor("e_up", [E, D], FP, kind="ExternalInput")
    id_d = nc.dram_tensor("ident", [128, 128], FP, kind="ExternalInput")
    io_d = nc.dram_tensor("iota64", [128, 64], FP, kind="ExternalInput")
    out_d = nc.dram_tensor("out", [TOK, D], FP, kind="ExternalOutput")

    with tile.TileContext(nc) as tc:
        with (
            tc.tile_pool(name="const", bufs=1) as cpool,
            tc.tile_pool(name="mats", bufs=1) as mpool,
            tc.tile_pool(name="route", bufs=3) as rpool,
            tc.tile_pool(name="persist", bufs=1) as ppool,
            tc.tile_pool(name="gd", bufs=4) as gdpool,
            tc.tile_pool(name="gu", bufs=8) as gupool,
            tc.tile_pool(name="scr", bufs=2) as spool,
            tc.tile_pool(name="dg", bufs=4) as dgpool,
            tc.tile_pool(name="ob", bufs=2) as opool,
            tc.tile_pool(name="psqt", bufs=2, space="PSUM") as psqt,
            tc.tile_pool(name="pssim", bufs=2, space="PSUM") as pssim,
            tc.tile_pool(name="psacc", bufs=1, space="PSUM") as psacc,
        ):
            wq_sb = mpool.tile([128, NDCH * 512], FP)
            MW = NDCH * 128  # 2048 columns per m-chunk of wq (m-major)
            # consts + the m=0 wq slice + xt load first; x and the remaining
            # wq slices trail BEHIND the first gathers so the DMA queues
            # never idle between the load phase and the gather stream.
            ident = cpool.tile([128, 128], FP)
            nc.sync.dma_start(ident[:], id_d.ap())
            iota = cpool.tile([128, 64], FP)
            nc.sync.dma_start(iota[:], io_d.ap())
            kt_sb = cpool.tile([2 * DK, H * NK], FP)
            nc.sync.dma_start(kt_sb[:], kt_d.ap())
            nc.sync.dma_start(wq_sb[:, 0:MW], wqs_d.ap()[:, 0:MW])
            xt_sb = mpool.tile([128, NDCH * TOK], FP)
            # quartered so qT chunk matmuls overlap the load stream
            for q4 in range(4):
                nc.sync.dma_start(
                    xt_sb[:, q4 * 4 * TOK : (q4 + 1) * 4 * TOK],
                    xts_d.ap()[:, q4 * 4 * TOK : (q4 + 1) * 4 * TOK],
                )

            # PE warmup: back-to-back matmuls on the first wq slice release
            # the HAM clock throttle while the big loads stream in, so the
            # latency-critical qT chain runs at full clock.
            wps = pssim.tile([128, 128], FP, tag="ps", name="warm")
            for w in range(12):
                nc.tensor.matmul(
                    wps[:, 0:128], lhsT=wq_sb[:, 0:128], rhs=wq_sb[:, 128:256],
                    start=(w == 0), stop=(w == 11),
                )
            wsb = rpool.tile([128, 128], FP, tag="sim", name="warmout")
            nc.scalar.activation(wsb[:], wps[:, 0:128], mybir.ActivationFunctionType.Copy)

            # trailing loads (consumed from the first consume chain on)
            x_sb = []
            for hf in range(2):
                xh = ppool.tile([128, D], FP, tag=f"x{hf}", name=f"x{hf}")
                nc.sync.dma_start(xh[:], x_d.ap()[hf * 128 : hf * 128 + 128, :])
                x_sb.append(xh)
            for m in range(1, 4):
                nc.sync.dma_start(
                    wq_sb[:, m * MW : (m + 1) * MW], wqs_d.ap()[:, m * MW : (m + 1) * MW]
                )

            qt_sb = ppool.tile([128, 4 * TOK], FP)
            fi_all = [
                ppool.tile([128, NSLOT], I32, tag=f"fi{hf}", name=f"fi{hf}")
                for hf in range(2)
            ]
            fsr_all = [
                ppool.tile([128, NSLOT], FP, tag=f"fsr{hf}", name=f"fsr{hf}")
                for hf in range(2)
            ]
            hid_all = [
                ppool.tile([128, NSLOT], FP, tag=f"hid{hf}", name=f"hid{hf}")
                for hf in range(2)
            ]
            hs2_all = [
                ppool.tile([128, NSLOT], FP, tag=f"hs2{hf}", name=f"hs2{hf}")
                for hf in range(2)
            ]
            acc = {}
            gts = {}

            def emit_qt(m):
                # qT chunk m: psum_q[p, t] = q[t, m*128+p] for all 256 tokens
                pq = psqt.tile([128, TOK], FP, tag="pq", name=f"pq{m}")
                for c in range(NDCH):
                    nc.tensor.matmul(
                        pq[:],
                        lhsT=wq_sb[:, m * MW + c * 128 : m * MW + (c + 1) * 128],
                        rhs=xt_sb[:, c * TOK : (c + 1) * TOK],
                        start=(c == 0),
                        stop=(c == NDCH - 1),
                    )
                nc.scalar.activation(
                    qt_sb[:, m * TOK : (m + 1) * TOK],
                    pq[:],
                    mybir.ActivationFunctionType.Copy,
                )

            def emit_route(hf, h):
                t0 = hf * 128
                ss = []
                ii = []
                for sub in range(2):
                    ps = pssim.tile([128, NK], FP, tag="ps")
                    nc.tensor.matmul(
                        ps[:],
                        lhsT=qt_sb[
                            sub * 64 : (sub + 1) * 64,
                            h * TOK + t0 : h * TOK + t0 + 128,
                        ],
                        rhs=kt_sb[sub * 64 : (sub + 1) * 64, h * NK : (h + 1) * NK],
                        start=True,
                        stop=True,
                    )
                    sim = rpool.tile([128, NK], FP, tag="sim")
                    nc.scalar.activation(
                        sim[:], ps[:], mybir.ActivationFunctionType.Copy
                    )
                    s = rpool.tile([128, 8], FP, tag="s")
                    nc.vector.max(s[:], sim[:])
                    idx = rpool.tile([128, 8], U32, tag="idx")
                    nc.vector.max_index(idx[:], s[:], sim[:])
                    idf = rpool.tile([128, 8], FP, tag="idf")
                    nc.vector.tensor_copy(idf[:], idx[:])
                    ss.append(s)
                    ii.append(idf)
                # cross combine: [128, 8(k1), 8(k2)]
                alls = rpool.tile([128, 64], FP, tag="alls")
                a3 = _re(alls[:], [[8, 8], [1, 8]])
                nc.vector.tensor_tensor(
                    out=a3,
                    in0=_re(ss[0][:], [[1, 8], [0, 8]]),
                    in1=_re(ss[1][:], [[0, 8], [1, 8]]),
                    op=mybir.AluOpType.add,
                )
                alli = rpool.tile([128, 64], FP, tag="alli")
                ai3 = _re(alli[:], [[8, 8], [1, 8]])
                nc.vector.tensor_scalar(
                    out=ai3,
                    in0=_re(ii[0][:], [[1, 8], [0, 8]]),
                    scalar1=float(NK),
                    scalar2=None,
                    op0=mybir.AluOpType.mult,
                )
                nc.vector.tensor_tensor(
                    out=ai3,
                    in0=ai3,
                    in1=_re(ii[1][:], [[0, 8], [1, 8]]),
                    op=mybir.AluOpType.add,
                )
                fs = rpool.tile([128, 8], FP, tag="fs")
                nc.vector.max(fs[:], alls[:])
                pk = rpool.tile([128, 8], U32, tag="pk")
                nc.vector.max_index(pk[:], fs[:], alls[:])
                pkf = rpool.tile([128, 8], FP, tag="pkf")
                nc.vector.tensor_copy(pkf[:], pk[:])
                # scores: relu on scalar engine
                nc.scalar.activation(
                    fsr_all[hf][:, h * 8 : (h + 1) * 8],
                    fs[:],
                    mybir.ActivationFunctionType.Relu,
                )
                # mask[p, j, n] = (pk[p,j] == iota[p,n]) * alli[p,n]; reduce
                mask = rpool.tile([128, 512], FP, tag="mask", bufs=2)
                m3 = _re(mask[:], [[64, 8], [1, 64]])
                nc.vector.tensor_tensor(
                    out=m3,
                    in0=_re(pkf[:], [[1, 8], [0, 64]]),
                    in1=_re(iota[:], [[0, 8], [1, 64]]),
                    op=mybir.AluOpType.is_equal,
                )
                nc.vector.tensor_tensor(
                    out=m3,
                    in0=m3,
                    in1=_re(alli[:], [[0, 8], [1, 64]]),
                    op=mybir.AluOpType.mult,
                )
                fif = rpool.tile([128, 8], FP, tag="fif")
                nc.vector.tensor_reduce(
                    fif[:],
                    m3,
                    axis=mybir.AxisListType.X,
                    op=mybir.AluOpType.add,
                )
                nc.vector.tensor_copy(fi_all[hf][:, h * 8 : (h + 1) * 8], fif[:])

            def emit_gather(hf, h):
                # For the very last chain, issue all down-gathers first: the
                # dot/silu chain then finishes while the up rows stream in,
                # so the exposed tail is just the final matmul + store.
                down_first = hf == 1 and h == H - 1
                gds = []
                for j in range(K):
                    k = h * 8 + j
                    gd = gdpool.tile([128, D], FP, tag="gd")
                    nc.gpsimd.indirect_dma_start(
                        out=gd[:],
                        out_offset=None,
                        in_=ed_d.ap(),
                        in_offset=bass.IndirectOffsetOnAxis(
                            ap=fi_all[hf][:, k : k + 1], axis=0
                        ),
                    )
                    gds.append(gd)
                    if not down_first:
                        gu = gupool.tile([128, D], FP, tag="gu")
                        nc.gpsimd.indirect_dma_start(
                            out=gu[:],
                            out_offset=None,
                            in_=eu_d.ap(),
                            in_offset=bass.IndirectOffsetOnAxis(
                                ap=fi_all[hf][:, k : k + 1], axis=0
                            ),
                        )
                        gts.setdefault((hf, h), []).append((gd, gu))
                if down_first:
                    for j in range(K):
                        k = h * 8 + j
                        gu = gupool.tile([128, D], FP, tag="gu")
                        nc.gpsimd.indirect_dma_start(
                            out=gu[:],
                            out_offset=None,
                            in_=eu_d.ap(),
                            in_offset=bass.IndirectOffsetOnAxis(
                                ap=fi_all[hf][:, k : k + 1], axis=0
                            ),
                        )
                        gts.setdefault((hf, h), []).append((gds[j], gu))

            def emit_consume(hf, h):
                t0 = hf * 128
                if h == 0:
                    acc[hf] = psacc.tile([128, D], FP, tag="acc", name=f"acc{hf}")
                # sub-groups of 2 slots: dots stream on DVE (scalar accums
                # trail by one slot), then a small silu/hs2 batch and the
                # dg+matmuls, so gather buffers release continuously and the
                # final group drains fast at the end of the stream.  The very
                # last chain ends in two 1-slot groups to minimize the tail.
                last = hf == 1 and h == H - 1
                groups = [(0, 2), (2, 2), (4, 2), (6, 1), (7, 1)] if last else [
                    (0, 2), (2, 2), (4, 2), (6, 2)
                ]
                for j0, glen in groups:
                    for j in range(j0, j0 + glen):
                        k = h * 8 + j
                        gd, gu = gts[(hf, h)][j]
                        scr = spool.tile([128, D], FP, tag="scr", bufs=3)
                        nc.vector.tensor_tensor(
                            out=scr[:],
                            in0=gd[:],
                            in1=x_sb[hf][:],
                            op=mybir.AluOpType.mult,
                        )
                        # in-place: the elementwise Copy output is discarded,
                        # only accum_out matters, so write it over the input
                        nc.scalar.activation(
                            scr[:],
                            scr[:],
                            mybir.ActivationFunctionType.Copy,
                            accum_out=hid_all[hf][:, k : k + 1],
                        )
                    k0 = h * 8 + j0
                    hsil4 = rpool.tile([128, 2], FP, tag="hsil", padded_shape=[128, 2])
                    nc.scalar.activation(
                        hsil4[:, 0:glen],
                        hid_all[hf][:, k0 : k0 + glen],
                        mybir.ActivationFunctionType.Silu,
                    )
                    nc.vector.tensor_tensor(
                        out=hs2_all[hf][:, k0 : k0 + glen],
                        in0=hsil4[:, 0:glen],
                        in1=fsr_all[hf][:, k0 : k0 + glen],
                        op=mybir.AluOpType.mult,
                    )
                    for j in range(j0, j0 + glen):
                        k = h * 8 + j
                        gd, gu = gts[(hf, h)][j]
                        dg = dgpool.tile([128, 128], FP, tag="dg")
                        nc.vector.tensor_scalar_mul(
                            dg[:], ident[:], hs2_all[hf][:, k : k + 1]
                        )
                        for c4 in range(4):
                            nc.tensor.matmul(
                                acc[hf][:, c4 * 512 : (c4 + 1) * 512],
                                lhsT=dg[:],
                                rhs=gu[:, c4 * 512 : (c4 + 1) * 512],
                                start=(k == 0),
                                stop=(k == NSLOT - 1),
                            )
                if h == H - 1:
                    # chunked copy-out overlaps the tail matmuls and halves
                    # the exposed drain at the end of each half.
                    for c4 in range(4):
                        obc = opool.tile([128, 512], FP, tag="obc")
                        nc.scalar.activation(
                            obc[:],
                            acc[hf][:, c4 * 512 : (c4 + 1) * 512],
                            mybir.ActivationFunctionType.Copy,
                        )
                        nc.sync.dma_start(
                            out_d.ap()[t0 : t0 + 128, c4 * 512 : (c4 + 1) * 512],
                            obc[:],
                        )

            # Chains c=0..7 -> (hf, h) = (c // 4, c % 4).  Routing stays one
            # chain ahead of consumption; gathers are enqueued early and
            # self-pace against gather-buffer releases (nothing else runs on
            # gpsimd, so SWDGE stalls are harmless).  qt chunks sit in PE
            # idle gaps one full window before the routing that needs them.
            CH = [(0, 0), (0, 1), (0, 2), (0, 3), (1, 0), (1, 1), (1, 2), (1, 3)]
            emit_qt(0)
            emit_route(*CH[0])
            emit_gather(*CH[0])
            emit_qt(1)
            emit_route(*CH[1])
            emit_gather(*CH[1])
            emit_qt(2)
            emit_route(*CH[2])
            emit_gather(*CH[2])
            emit_consume(*CH[0])
            emit_qt(3)
            emit_route(*CH[3])
            emit_gather(*CH[3])
            emit_consume(*CH[1])
            emit_route(*CH[4])
            emit_gather(*CH[4])
            emit_consume(*CH[2])
            emit_route(*CH[5])
            emit_gather(*CH[5])
            emit_consume(*CH[3])
            emit_route(*CH[6])
            emit_gather(*CH[6])
            emit_consume(*CH[4])
            emit_route(*CH[7])
            emit_gather(*CH[7])
            emit_consume(*CH[5])
            emit_consume(*CH[6])
            emit_consume(*CH[7])

    return nc


_CACHED = {}


def kernel(x, Wq, keys, e_down, e_up):
    x = np.asarray(x, dtype=np.float32)
    Wq = np.asarray(Wq, dtype=np.float32)
    keys = np.asarray(keys, dtype=np.float32)
    e_down = np.asarray(e_down, dtype=np.float32)
    e_up = np.asarray(e_up, dtype=np.float32)

    if "nc" not in _CACHED:
        _CACHED["nc"] = build_program()
    nc = _CACHED["nc"]

    xf = x.reshape(B * T, D)
    keyst = np.ascontiguousarray(keys.transpose(2, 3, 0, 1)).reshape(2 * DK, H * NK)
    # keyst[sub*64+dk, h*NK + nk] = keys[h, nk, sub, dk]
    ident = np.eye(128, dtype=np.float32)
    iota64 = np.tile(np.arange(64, dtype=np.float32), (128, 1))
    NDCH = D // 128
    # wqs[p, m*16*128 + c*128 + o] = Wq[c*128+p, m*128+o] (m-major)
    wqs = np.ascontiguousarray(
        Wq.reshape(NDCH, 128, 4, 128).transpose(1, 2, 0, 3).reshape(128, NDCH * 512)
    )

    in_maps = []
    for c in range(N_CORES):
        xs = np.ascontiguousarray(xf[c * TOK : (c + 1) * TOK])
        # xts[p, ch*TOK+t] = xs[t, ch*128+p]
        xts = np.ascontiguousarray(
            xs.reshape(TOK, NDCH, 128).transpose(2, 1, 0).reshape(128, NDCH * TOK)
        )
        in_maps.append(
            {
                "x": xs,
                "xts": xts,
                "wqs": wqs,
                "keyst": keyst,
                "e_down": e_down,
                "e_up": e_up,
                "ident": ident,
                "iota64": iota64,
            }
        )

    res = run_bass_kernel_spmd(nc, in_maps, core_ids=list(range(N_CORES)))
    _CACHED["res"] = res
    out = np.concatenate([res.results[c]["out"] for c in range(N_CORES)], axis=0)
    return out.reshape(B, T, D)


# revision 38
# speedup vs baseline: 1.1287x; 1.1226x over previous
"""LlamaPEER MoE-routing kernel for 8 NeuronCores (TRN2, Bass/Tile).

Data-parallel over B*T (2048 tokens -> 256/core); expert tables replicated
as one host-packed fp32 [E, 2D] table (down|up rows concatenated), so each
(token, slot) needs ONE 16KB-descriptor indirect gather. Per core:
  1. qT = Wq^T @ x^T on PE (fp32, routing bit-identical to reference).
  2. Per (half, head) chain: sims on PE, top-8 via DVE max/max_index,
     K x K cross combine + second top-8 + iota-mask index extraction.
  3. Per slot: gather cat row; down-dot via DVE mult + Scalar accumulate;
     silu (Scalar) * relu(score); diag build (DVE); up-proj via PE diag
     matmuls accumulated in PSUM; one copy-out + store per 128-token half.
Emission interleaves routing/gather/consume chains so the gather stream
starts ~40us in and the 16 DMA queues stay saturated.
"""

import numpy as np

import concourse.bass as bass
import concourse.tile as tile
from concourse import mybir
from concourse.bass_utils import run_bass_kernel_spmd
from concourse.vector_clock import ScopedClock

N_CORES = 8
B, T, D = 2, 1024, 2048
H, K, DK = 4, 8, 64
E = 16384
NK = 128
TOK = (B * T) // N_CORES  # 256 tokens per core
NSLOT = H * K  # 32 slots per 128-token half
CAT = 2 * D  # concatenated row length
FP = mybir.dt.float32
I32 = mybir.dt.int32
U32 = mybir.dt.uint32

# --- workaround: this walrus build allows only 1 sync-wait command on the
# final SP drain; split the tile-context drain into 1-wait drains.
_MAX_DRAIN_WAITS = 1


def _patched_drain_and_barrier(self, tick_clock, wait_clock):
    nc = self.nc
    drain_inst = nc.sync.drain()
    wait_clock.add_sem_waits(
        drain_inst.ins, ScopedClock({None: tick_clock.global_clock})
    )
    si = drain_inst.ins.sync_info
    if si is not None and len(si.on_wait) > _MAX_DRAIN_WAITS:
        waits = list(si.on_wait)
        upds = list(si.on_update)
        drain_inst.ins.sync_info = mybir.SyncInfo(
            on_wait=waits[:_MAX_DRAIN_WAITS], on_update=[]
        )
        rest = waits[_MAX_DRAIN_WAITS:]
        while rest:
            extra = nc.sync.drain()
            extra.ins.sync_info = mybir.SyncInfo(
                on_wait=rest[:_MAX_DRAIN_WAITS],
                on_update=upds if len(rest) <= _MAX_DRAIN_WAITS else [],
            )
            rest = rest[_MAX_DRAIN_WAITS:]
    nc.all_engine_barrier()
    popped = nc._tile_sem_poison_stack.pop()
    assert popped is self._sem_poison
    all_sems = list(self.sems.allocated().values())
    for i in range(0, len(all_sems), 8):
        nc.clear_and_free_semaphores(all_sems[i : i + 8])
    nc.all_engine_barrier()


tile.TileContext._drain_and_barrier = _patched_drain_and_barrier

_orig_lower_ordered = tile.TileContext._lower_ordered_insts


def _patched_lower_ordered(self, postordered_blocks):
    # this walrus build supports only one sync-wait command per instruction:
    # hoist extra waits onto same-engine NoOps placed just before.
    for bb_name, insts in postordered_blocks.items():
        new = []
        for inst in insts:
            si = getattr(inst, "sync_info", None)
            eng = getattr(inst, "engine", None)
            if si is not None and eng is not None and len(si.on_wait) > 1:
                waits = list(si.on_wait)
                for w in waits[:-1]:
                    nop = mybir.InstNoOp(
                        name=self.nc.get_next_instruction_name(),
                        sync_info=mybir.SyncInfo(on_wait=[w], on_update=[]),
                        bass_nofuse=True,
                        engine=eng,
                    )
                    new.append(nop)
                inst.sync_info = mybir.SyncInfo(
                    on_wait=[waits[-1]], on_update=list(si.on_update)
                )
            new.append(inst)
        insts[:] = new
    return _orig_lower_ordered(self, postordered_blocks)


tile.TileContext._lower_ordered_insts = _patched_lower_ordered


def _re(ap, dims):
    """Return ap with its free-axis access pattern replaced by `dims`
    (list of [step, count]); keeps the partition dim."""
    return ap.__replace__(ap=[list(ap.ap)[0]] + [list(d) for d in dims])


def build_program():
    nc = bass.Bass("TRN2", target_bir_lowering=False, debug=False)

    NDCH = D // 128  # 16 d-chunks

    # xts/wqs are host-prelayouted to the exact SBUF image (contiguous
    # 16KB-per-partition loads): xts[p, c*TOK+t] = x[t, c*128+p];
    # wqs[p, m*16*128 + c*128 + o] = Wq[c*128+p, m*128+o] (m-major, so the
    # 1MB slice feeding qT chunk m=0 loads first and routing starts early).
    xts_d = nc.dram_tensor("xts", [128, NDCH * TOK], FP, kind="ExternalInput")
    x_d = nc.dram_tensor("x", [TOK, D], FP, kind="ExternalInput")
    wqs_d = nc.dram_tensor("wqs", [128, NDCH * 512], FP, kind="ExternalInput")
    kt_d = nc.dram_tensor("keyst", [2 * DK, H * NK], FP, kind="ExternalInput")
    ed_d = nc.dram_tensor("e_down", [E, D], FP, kind="ExternalInput")
    eu_d = nc.dram_tensor("e_up", [E, D], FP, kind="ExternalInput")
    id_d = nc.dram_tensor("ident", [128, 128], FP, kind="ExternalInput")
    io_d = nc.dram_tensor("iota64", [128, 64], FP, kind="ExternalInput")
    out_d = nc.dram_tensor("out", [TOK, D], FP, kind="ExternalOutput")

    with tile.TileContext(nc) as tc:
        with (
            tc.tile_pool(name="const", bufs=1) as cpool,
            tc.tile_pool(name="mats", bufs=1) as mpool,
            tc.tile_pool(name="route", bufs=3) as rpool,
            tc.tile_pool(name="persist", bufs=1) as ppool,
            tc.tile_pool(name="gd", bufs=3) as gdpool,
            tc.tile_pool(name="gu", bufs=8) as gupool,
            tc.tile_pool(name="scr", bufs=2) as spool,
            tc.tile_pool(name="dg", bufs=4) as dgpool,
            tc.tile_pool(name="ob", bufs=2) as opool,
            tc.tile_pool(name="psqt", bufs=2, space="PSUM") as psqt,
            tc.tile_pool(name="pssim", bufs=2, space="PSUM") as pssim,
            tc.tile_pool(name="psacc", bufs=1, space="PSUM") as psacc,
        ):
            wq_sb = mpool.tile([128, NDCH * 512], FP)
            MW = NDCH * 128  # 2048 columns per m-chunk of wq (m-major)
            # consts + the m=0 wq slice + xt load first; x and the remaining
            # wq slices trail BEHIND the first gathers so the DMA queues
            # never idle between the load phase and the gather stream.
            ident = cpool.tile([128, 128], FP)
            nc.sync.dma_start(ident[:], id_d.ap())
            iota = cpool.tile([128, 64], FP)
            nc.sync.dma_start(iota[:], io_d.ap())
            kt_sb = cpool.tile([2 * DK, H * NK], FP)
            nc.sync.dma_start(kt_sb[:], kt_d.ap())
            nc.sync.dma_start(wq_sb[:, 0:MW], wqs_d.ap()[:, 0:MW])
            xt_sb = mpool.tile([128, NDCH * TOK], FP)
            # quartered so qT chunk matmuls overlap the load stream
            for q4 in range(4):
                nc.sync.dma_start(
                    xt_sb[:, q4 * 4 * TOK : (q4 + 1) * 4 * TOK],
                    xts_d.ap()[:, q4 * 4 * TOK : (q4 + 1) * 4 * TOK],
                )

            # PE warmup: back-to-back matmuls on the first wq slice release
            # the HAM clock throttle while the big loads stream in, so the
            # latency-critical qT chain runs at full clock.
            wps = pssim.tile([128, 128], FP, tag="ps", name="warm")
            for w in range(12):
                nc.tensor.matmul(
                    wps[:, 0:128], lhsT=wq_sb[:, 0:128], rhs=wq_sb[:, 128:256],
                    start=(w == 0), stop=(w == 11),
                )
            wsb = rpool.tile([128, 128], FP, tag="sim", name="warmout")
            nc.scalar.activation(wsb[:], wps[:, 0:128], mybir.ActivationFunctionType.Copy)

            # trailing loads (consumed from the first consume chain on)
            x_sb = []
            for hf in range(2):
                xh = ppool.tile([128, D], FP, tag=f"x{hf}", name=f"x{hf}")
                nc.sync.dma_start(xh[:], x_d.ap()[hf * 128 : hf * 128 + 128, :])
                x_sb.append(xh)
            for m in range(1, 4):
                nc.sync.dma_start(
                    wq_sb[:, m * MW : (m + 1) * MW], wqs_d.ap()[:, m * MW : (m + 1) * MW]
                )

            qt_sb = ppool.tile([128, 4 * TOK], FP)
            fi_all = [
                ppool.tile([128, NSLOT], I32, tag=f"fi{hf}", name=f"fi{hf}")
                for hf in range(2)
            ]
            fsr_all = [
                ppool.tile([128, NSLOT], FP, tag=f"fsr{hf}", name=f"fsr{hf}")
                for hf in range(2)
            ]
            hid_all = [
                ppool.tile([128, NSLOT], FP, tag=f"hid{hf}", name=f"hid{hf}")
                for hf in range(2)
            ]
            hs2_all = [
                ppool.tile([128, NSLOT], FP, tag=f"hs2{hf}", name=f"hs2{hf}")
                for hf in range(2)
            ]
            acc = {}
            gts = {}

            def emit_qt(m):
                # qT chunk m: psum_q[p, t] = q[t, m*128+p] for all 256 tokens
                pq = psqt.tile([128, TOK], FP, tag="pq", name=f"pq{m}")
                for c in range(NDCH):
                    nc.tensor.matmul(
                        pq[:],
                        lhsT=wq_sb[:, m * MW + c * 128 : m * MW + (c + 1) * 128],
                        rhs=xt_sb[:, c * TOK : (c + 1) * TOK],
                        start=(c == 0),
                        stop=(c == NDCH - 1),
                    )
                nc.scalar.activation(
                    qt_sb[:, m * TOK : (m + 1) * TOK],
                    pq[:],
                    mybir.ActivationFunctionType.Copy,
                )

            def emit_route(hf, h):
                t0 = hf * 128
                ss = []
                ii = []
                for sub in range(2):
                    ps = pssim.tile([128, NK], FP, tag="ps")
                    nc.tensor.matmul(
                        ps[:],
                        lhsT=qt_sb[
                            sub * 64 : (sub + 1) * 64,
                            h * TOK + t0 : h * TOK + t0 + 128,
                        ],
                        rhs=kt_sb[sub * 64 : (sub + 1) * 64, h * NK : (h + 1) * NK],
                        start=True,
                        stop=True,
                    )
                    sim = rpool.tile([128, NK], FP, tag="sim")
                    nc.scalar.activation(
                        sim[:], ps[:], mybir.ActivationFunctionType.Copy
                    )
                    s = rpool.tile([128, 8], FP, tag="s")
                    nc.vector.max(s[:], sim[:])
                    idx = rpool.tile([128, 8], U32, tag="idx")
                    nc.vector.max_index(idx[:], s[:], sim[:])
                    idf = rpool.tile([128, 8], FP, tag="idf")
                    nc.vector.tensor_copy(idf[:], idx[:])
                    ss.append(s)
                    ii.append(idf)
                # cross combine: [128, 8(k1), 8(k2)]
                alls = rpool.tile([128, 64], FP, tag="alls")
                a3 = _re(alls[:], [[8, 8], [1, 8]])
                nc.vector.tensor_tensor(
                    out=a3,
                    in0=_re(ss[0][:], [[1, 8], [0, 8]]),
                    in1=_re(ss[1][:], [[0, 8], [1, 8]]),
                    op=mybir.AluOpType.add,
                )
                alli = rpool.tile([128, 64], FP, tag="alli")
                ai3 = _re(alli[:], [[8, 8], [1, 8]])
                nc.vector.tensor_scalar(
                    out=ai3,
                    in0=_re(ii[0][:], [[1, 8], [0, 8]]),
                    scalar1=float(NK),
                    scalar2=None,
                    op0=mybir.AluOpType.mult,
                )
                nc.vector.tensor_tensor(
                    out=ai3,
                    in0=ai3,
                    in1=_re(ii[1][:], [[0, 8], [1, 8]]),
                    op=mybir.AluOpType.add,
                )
                fs = rpool.tile([128, 8], FP, tag="fs")
                nc.vector.max(fs[:], alls[:])
                pk = rpool.tile([128, 8], U32, tag="pk")
                nc.vector.max_index(pk[:], fs[:], alls[:])
                pkf = rpool.tile([128, 8], FP, tag="pkf")
                nc.vector.tensor_copy(pkf[:], pk[:])
                # scores: relu on scalar engine
                nc.scalar.activation(
                    fsr_all[hf][:, h * 8 : (h + 1) * 8],
                    fs[:],
                    mybir.ActivationFunctionType.Relu,
                )
                # mask[p, j, n] = (pk[p,j] == iota[p,n]) * alli[p,n]; reduce
                mask = rpool.tile([128, 512], FP, tag="mask", bufs=2)
                m3 = _re(mask[:], [[64, 8], [1, 64]])
                nc.vector.tensor_tensor(
                    out=m3,
                    in0=_re(pkf[:], [[1, 8], [0, 64]]),
                    in1=_re(iota[:], [[0, 8], [1, 64]]),
                    op=mybir.AluOpType.is_equal,
                )
                nc.vector.tensor_tensor(
                    out=m3,
                    in0=m3,
                    in1=_re(alli[:], [[0, 8], [1, 64]]),
                    op=mybir.AluOpType.mult,
                )
                fif = rpool.tile([128, 8], FP, tag="fif")
                nc.vector.tensor_reduce(
                    fif[:],
                    m3,
                    axis=mybir.AxisListType.X,
                    op=mybir.AluOpType.add,
                )
                nc.vector.tensor_copy(fi_all[hf][:, h * 8 : (h + 1) * 8], fif[:])

            def emit_gather(hf, h):
                # For the very last chain, issue all down-gathers first: the
                # dot/silu chain then finishes while the up rows stream in,
                # so the exposed tail is just the final matmul + store.
                down_first = hf == 1 and h == H - 1
                gds = []
                for j in range(K):
                    k = h * 8 + j
                    gd = gdpool.tile([128, D], FP, tag="gd")
                    nc.gpsimd.indirect_dma_start(
                        out=gd[:],
                        out_offset=None,
                        in_=ed_d.ap(),
                        in_offset=bass.IndirectOffsetOnAxis(
                            ap=fi_all[hf][:, k : k + 1], axis=0
                        ),
                    )
                    gds.append(gd)
                    if not down_first:
                        gu = gupool.tile([128, D], FP, tag="gu")
                        nc.gpsimd.indirect_dma_start(
                            out=gu[:],
                            out_offset=None,
                            in_=eu_d.ap(),
                            in_offset=bass.IndirectOffsetOnAxis(
                                ap=fi_all[hf][:, k : k + 1], axis=0
                            ),
                        )
                        gts.setdefault((hf, h), []).append((gd, gu))
                if down_first:
                    for j in range(K):
                        k = h * 8 + j
                        gu = gupool.tile([128, D], FP, tag="gu")
                        nc.gpsimd.indirect_dma_start(
                            out=gu[:],
                            out_offset=None,
                            in_=eu_d.ap(),
                            in_offset=bass.IndirectOffsetOnAxis(
                                ap=fi_all[hf][:, k : k + 1], axis=0
                            ),
                        )
                        gts.setdefault((hf, h), []).append((gds[j], gu))

            def emit_consume(hf, h):
                t0 = hf * 128
                if h == 0:
                    acc[hf] = psacc.tile([128, D], FP, tag="acc", name=f"acc{hf}")
                # sub-groups of 2 slots: dots stream on DVE (scalar accums
                # trail by one slot), then a small silu/hs2 batch and the
                # dg+matmuls, so gather buffers release continuously and the
                # final group drains fast at the end of the stream.  The very
                # last chain ends in two 1-slot groups to minimize the tail.
                last = hf == 1 and h == H - 1
                groups = [(0, 2), (2, 2), (4, 2), (6, 1), (7, 1)] if last else [
                    (0, 2), (2, 2), (4, 2), (6, 2)
                ]
                for j0, glen in groups:
                    for j in range(j0, j0 + glen):
                        k = h * 8 + j
                        gd, gu = gts[(hf, h)][j]
                        scr = spool.tile([128, D], FP, tag="scr", bufs=3)
                        nc.vector.tensor_tensor(
                            out=scr[:],
                            in0=gd[:],
                            in1=x_sb[hf][:],
                            op=mybir.AluOpType.mult,
                        )
                        scr2 = spool.tile([128, D], FP, tag="scr2", bufs=1)
                        nc.scalar.activation(
                            scr2[:],
                            scr[:],
                            mybir.ActivationFunctionType.Copy,
                            accum_out=hid_all[hf][:, k : k + 1],
                        )
                    k0 = h * 8 + j0
                    hsil4 = rpool.tile([128, 2], FP, tag="hsil", padded_shape=[128, 2])
                    nc.scalar.activation(
                        hsil4[:, 0:glen],
                        hid_all[hf][:, k0 : k0 + glen],
                        mybir.ActivationFunctionType.Silu,
                    )
                    nc.vector.tensor_tensor(
                        out=hs2_all[hf][:, k0 : k0 + glen],
                        in0=hsil4[:, 0:glen],
                        in1=fsr_all[hf][:, k0 : k0 + glen],
                        op=mybir.AluOpType.mult,
                    )
                    for j in range(j0, j0 + glen):
                        k = h * 8 + j
                        gd, gu = gts[(hf, h)][j]
                        dg = dgpool.tile([128, 128], FP, tag="dg")
                        nc.vector.tensor_scalar_mul(
                            dg[:], ident[:], hs2_all[hf][:, k : k + 1]
                        )
                        for c4 in range(4):
                            nc.tensor.matmul(
                                acc[hf][:, c4 * 512 : (c4 + 1) * 512],
                                lhsT=dg[:],
                                rhs=gu[:, c4 * 512 : (c4 + 1) * 512],
                                start=(k == 0),
                                stop=(k == NSLOT - 1),
                            )
                if h == H - 1:
                    # chunked copy-out overlaps the tail matmuls and halves
                    # the exposed drain at the end of each half.
                    for c4 in range(4):
                        obc = opool.tile([128, 512], FP, tag="obc")
                        nc.scalar.activation(
                            obc[:],
                            acc[hf][:, c4 * 512 : (c4 + 1) * 512],
                            mybir.ActivationFunctionType.Copy,
                        )
                        nc.sync.dma_start(
                            out_d.ap()[t0 : t0 + 128, c4 * 512 : (c4 + 1) * 512],
                            obc[:],
                        )

            # Chains c=0..7 -> (hf, h) = (c // 4, c % 4).  Routing stays one
            # chain ahead of consumption; gathers are enqueued early and
            # self-pace against gather-buffer releases (nothing else runs on
            # gpsimd, so SWDGE stalls are harmless).  qt chunks sit in PE
            # idle gaps one full window before the routing that needs them.
            CH = [(0, 0), (0, 1), (0, 2), (0, 3), (1, 0), (1, 1), (1, 2), (1, 3)]
            emit_qt(0)
            emit_route(*CH[0])
            emit_gather(*CH[0])
            emit_qt(1)
            emit_route(*CH[1])
            emit_gather(*CH[1])
            emit_qt(2)
            emit_route(*CH[2])
            emit_gather(*CH[2])
            emit_consume(*CH[0])
            emit_qt(3)
            emit_route(*CH[3])
            emit_gather(*CH[3])
            emit_consume(*CH[1])
            emit_route(*CH[4])
            emit_gather(*CH[4])
            emit_consume(*CH[2])
            emit_route(*CH[5])
            emit_gather(*CH[5])
            emit_consume(*CH[3])
            emit_route(*CH[6])
            emit_gather(*CH[6])
            emit_consume(*CH[4])
            emit_route(*CH[7])
            emit_gather(*CH[7])
            emit_consume(*CH[5])
            emit_consume(*CH[6])
            emit_consume(*CH[7])

    return nc


_CACHED = {}


def kernel(x, Wq, keys, e_down, e_up):
    x = np.asarray(x, dtype=np.float32)
    Wq = np.asarray(Wq, dtype=np.float32)
    keys = np.asarray(keys, dtype=np.float32)
    e_down = np.asarray(e_down, dtype=np.float32)
    e_up = np.asarray(e_up, dtype=np.float32)

    if "nc" not in _CACHED:
        _CACHED["nc"] = build_program()
    nc = _CACHED["nc"]

    xf = x.reshape(B * T, D)
    keyst = np.ascontiguousarray(keys.transpose(2, 3, 0, 1)).reshape(2 * DK, H * NK)
    # keyst[sub*64+dk, h*NK + nk] = keys[h, nk, sub, dk]
    ident = np.eye(128, dtype=np.float32)
    iota64 = np.tile(np.arange(64, dtype=np.float32), (128, 1))
    NDCH = D // 128
    # wqs[p, m*16*128 + c*128 + o] = Wq[c*128+p, m*128+o] (m-major)
    wqs = np.ascontiguousarray(
        Wq.reshape(NDCH, 128, 4, 128).transpose(1, 2, 0, 3).reshape(128, NDCH * 512)
    )

    in_maps = []
    for c in range(N_CORES):
        xs = np.ascontiguousarray(xf[c * TOK : (c + 1) * TOK])
        # xts[p, ch*TOK+t] = xs[t, ch*128+p]
        xts = np.ascontiguousarray(
            xs.reshape(TOK, NDCH, 128).transpose(2, 1, 0).reshape(128, NDCH * TOK)
        )
        in_maps.append(
            {
                "x": xs,
                "xts": xts,
                "wqs": wqs,
                "keyst": keyst,
                "e_down": e_down,
                "e_up": e_up,
                "ident": ident,
                "iota64": iota64,
            }
        )

    res = run_bass_kernel_spmd(nc, in_maps, core_ids=list(range(N_CORES)))
    _CACHED["res"] = res
    out = np.concatenate([res.results[c]["out"] for c in range(N_CORES)], axis=0)
    return out.reshape(B, T, D)
